# revision 4
# baseline (speedup 1.0000x reference)
"""Trainium2 Bass kernel v8 for EpsilonNetGM score (8-core data parallel).

Math (closed-form score, no autodiff):
  acp = alphas_cumprod[t]; mu_k = sqrt(acp)*means_k
  Sigma_k = (1-acp) I + acp covs_k ; L = chol(Sigma); Linv = L^-1; P = Linv^T Linv
  z_k(x) = Linv_k x
  l_k(x) = -0.5|z_k|^2 + (P_k mu_k).x + c'_k      (c' <= 0, so exp never
  r~ = exp(l)  (UNNORMALIZED; l ~ -32 so no overflow/underflow risk)
  out = sqrt(1-acp)/S * [ sum_k Linv_k^T (r~_k z_k) - sum_k r~_k h_k ],
  S = sum_k r~_k

v8 key facts (measured on HW, not the CoreSim cost model):
  - fp32r matmul ~0.9ns/moving-row (full 128-partition out) + ~180ns fixed
    per instruction in-kernel; so NB=512 chunks halve instruction count
    and cut the PE stream 41% vs NB=256.
  - DVE tensor ops with BOTH operands in SBUF run ~2x faster (356ns/512el)
    than PSUM-reading ops (~650-720ns); Pool (gpsimd) works at ~1049ns.
  - So: z is computed once, evacuated PSUM->SBUF, squares and W=z*r are
    cheap SBUF-SBUF TTs spread across Pool/DVE, and there is no z
    recompute. W runs a full pipeline stage before its mm2.

Pipeline per emission iteration c (NCHUNK=16 chunks of 512 rows):
  back(c-2) PE first (zero-gated: wsb finished last iter), then mid(c-1)
  (exp^T/replicate/sum/recip + W), then front(c) waves (mm1 -> zevac ->
  square -> trailing maha), transpose prefetch (c+1), and back(c-2)
  finish (1/S scale + DMA) last.
"""

import math
import sys

import numpy as np

sys.path.insert(0, "/opt/trn_rl_repo")

import concourse.bass as bass  # noqa: E402
import concourse.tile as tile  # noqa: E402
from concourse import mybir  # noqa: E402
from concourse.bass_utils import run_bass_kernel_spmd  # noqa: E402

B, K, D, T = 65536, 16, 64, 1000
NCORES = 8
BP = B // NCORES          # rows per core = 8192
NB = 512                  # batch chunk (free dim)
NTILE = 4                 # 128-row tiles per chunk
NCHUNK = BP // NB         # 16
DS = 8                    # d-subtile width; partition p = 8*k + ds
NT = D // DS              # 8 subtiles
CW = 2432                 # const blob width

F32 = mybir.dt.float32
F32R = mybir.dt.float32r


def _host_precompute(means, weights, covs, alphas_cumprod, t):
    acp = float(np.asarray(alphas_cumprod)[int(t)])
    s1 = math.sqrt(acp)
    sqrt1m = math.sqrt(1.0 - acp)
    mu = (s1 * means).astype(np.float64)
    covs = covs.astype(np.float64)
    sigma = (1.0 - acp) * np.eye(D) + acp * covs
    chol = np.linalg.cholesky(sigma)
    Linv = np.stack([np.linalg.solve(chol[k], np.eye(D)) for k in range(K)])
    P = np.einsum("kdi,kdj->kij", Linv, Linv)
    h = np.einsum("kij,kj->ki", P, mu)
    logdet = 2.0 * np.log(np.diagonal(chol, axis1=1, axis2=2)).sum(-1)
    w = weights.astype(np.float64)
    logw = np.log(w) - math.log(w.sum())
    c = logw - 0.5 * (D * math.log(2 * math.pi) + logdet)
    cp = c - 0.5 * np.einsum("ki,ki->k", mu, h)
    cp = cp - cp.max()

    # A1s [128, NT, 128]: rows d' (dup 0-63/64-127), col p = 8k+ds
    A1 = np.zeros((64, NT, 128), dtype=np.float32)
    A2s = np.zeros((128, NT, 64), dtype=np.float32)
    for k in range(K):
        for ds in range(DS):
            p = 8 * k + ds
            for tt in range(NT):
                A1[:, tt, p] = Linv[k, 8 * tt + ds, :]
                A2s[p, tt, :] = sqrt1m * Linv[k, 8 * tt + ds, :]
    A1s = np.concatenate([A1, A1], axis=0)

    onesblk = np.zeros((128, K), dtype=np.float32)
    for k in range(K):
        onesblk[8 * k : 8 * k + 8, k] = -0.5  # fold -0.5 into the reduce
    cmm = cp.astype(np.float32).reshape(1, K)
    ERep = np.zeros((K, 128), dtype=np.float32)
    for k in range(K):
        ERep[k, 8 * k : 8 * k + 8] = 1.0
    negHs = (-sqrt1m * h).astype(np.float32)   # [K, 64]
    H2c = h.T.astype(np.float32)               # [64, K]
    ident = np.eye(128, dtype=np.float32)

    blob = np.zeros((128, CW), dtype=np.float32)
    blob[:, 0:1024] = A1s.reshape(128, 1024)
    blob[:, 1024:1536] = A2s.reshape(128, 512)
    blob[:, 1536:1552] = onesblk
    blob[:, 1552:1680] = ident
    blob[0, 1680:1696] = cmm[0]
    blob[0:16, 1696:1824] = ERep
    blob[0:16, 1824:1888] = negHs
    blob[0:64, 1888:1904] = H2c
    blob[0, 1904 : 1904 + NB] = 1.0   # ones row [1, NB]
    blob[0:16, 2416] = 1.0            # ones column [16, 1] for the k-sum
    return dict(cblob=blob)


def _build_bass(nchunk=NCHUNK, repeat=None):
    """repeat=R wraps the chunk loop in a hardware For_i for timing builds."""
    import contextlib

    nc = bass.Bass()
    x_in = nc.declare_dram_parameter("x_in", [BP, D], F32R, isOutput=False)
    outT = nc.declare_dram_parameter("outT", [D, BP], F32, isOutput=True)
    c_blob = nc.declare_dram_parameter("cblob", [128, CW], F32R, isOutput=False)

    xv = x_in.rearrange("(n j p) d -> n p j d", p=128, j=NTILE)
    ovT = outT.rearrange("d (n b) -> n d b", b=NB)

    r = lambda ap: ap.bitcast(F32R)  # noqa: E731

    with tile.TileContext(nc) as tc:
        with (
            tc.tile_pool(name="consts", bufs=1) as consts,
            tc.tile_pool(name="xin", bufs=3) as xin_pool,
            tc.tile_pool(name="xts", bufs=2) as xts_pool,
            tc.tile_pool(name="zpsum", bufs=2, space="PSUM") as zpsum,
            tc.tile_pool(name="spsum", bufs=1, space="PSUM") as spsum,
            tc.tile_pool(name="sq", bufs=2) as sq_pool,
            tc.tile_pool(name="zsb", bufs=2) as z_pool,
            tc.tile_pool(name="small", bufs=3) as small_pool,
            tc.tile_pool(name="wbuf", bufs=2) as w_pool,
            tc.tile_pool(name="obuf", bufs=2) as o_pool,
        ):
            cblob = consts.tile([128, CW], F32R)
            nc.sync.dma_start(out=cblob, in_=c_blob[...])
            ct = {
                "A1s": cblob[:, 0:1024].rearrange("p (t c) -> p t c", t=NT),
                "A2s": cblob[:, 1024:1536].rearrange("p (t c) -> p t c", t=NT),
                "onesblk": cblob[:, 1536:1552],
                "ident": cblob[:, 1552:1680],
                "cmm": cblob[0:1, 1680:1696],
                "ERep": cblob[0:16, 1696:1824],
                "negHs": cblob[0:16, 1824:1888],
                "H2c": cblob[0:64, 1888:1904],
            }
            onesr = cblob[0:1, 1904 : 1904 + NB]
            ones64 = cblob[0:1, 1904 : 1904 + 64]
            ones16 = cblob[0:16, 2416:2417]

            # PE warmup read of cblob so later matmuls don't each need a
            # DMA wait (walrus allows only one sync-wait per instruction).
            pwarm = zpsum.tile([128, NB], F32, tag="z")
            nc.tensor.matmul(
                pwarm[0:32, 0:32].bitcast(F32R), ct["ident"][0:32, 0:32],
                ct["ident"][0:32, 0:32], is_transpose=True,
            )

            state = {}

            def emit_iter(c):
                has_front = c < nchunk
                has_tr = c + 1 < nchunk
                has_mid = 1 <= c <= nchunk
                has_back = c >= 2

                # single-buffered PSUM banks (one per role; all roles are
                # intra-iteration except pm which is read early next iter)
                pm = spsum.tile([K, NB], F32, tag="pm")
                psumS = spsum.tile([1, NB], F32, tag="psumS")
                pxt = spsum.tile([64, NB], F32, tag="pxt")
                po = spsum.tile([64, NB], F32, tag="po")
                prep = spsum.tile([128, NB], F32, tag="prep")
                sinvB = spsum.tile([64, NB], F32, tag="sinvB")

                # ---- back(c-2) PE section first: fully slack-fed ----
                if has_back:
                    stb = state.pop(c - 2)
                    eeTb, sinvb, wsbb = stb["eeT"], stb["sinv"], stb["wsb"]
                    nc.tensor.matmul(po, ct["negHs"], eeTb, start=True,
                                     stop=False)
                    for tt in range(NT):
                        nc.tensor.matmul(
                            po, ct["A2s"][:, tt, :], wsbb[:, tt, :],
                            start=False, stop=(tt == NT - 1),
                        )
                    nc.tensor.matmul(sinvB, ones64, sinvb, start=True,
                                     stop=True)

                # ---- mid(c-1): exp^T, replicate, k-sum, recip, W ----
                if has_mid:
                    stm = state[c - 1]
                    pmm = stm["pm"]
                    eeT = small_pool.tile([K, NB], F32R, tag="eeT")
                    nc.scalar.activation(
                        eeT, pmm,
                        mybir.ActivationFunctionType.Exp,
                        bias=0.0, scale=1.0,
                    )
                    rrepE = small_pool.tile([128, NB], F32, tag="rrepE")
                    nc.tensor.matmul(prep, ct["ERep"], eeT, start=True,
                                     stop=True)
                    nc.scalar.copy(out=rrepE, in_=prep)
                    nc.tensor.matmul(psumS, ones16, eeT, start=True,
                                     stop=True)
                    sinv = small_pool.tile([1, NB], F32R, tag="sinv")
                    with nc.allow_low_precision(
                        reason="f32r is full fp32 width; tag only gates PE"
                    ):
                        nc.vector.reciprocal(sinv, psumS)
                    stm["eeT"] = eeT
                    stm["sinv"] = sinv
                    # W(c-1) = zsb * r~rep: SBUF-SBUF TTs, one stage ahead
                    # of mm2; split DVE/Pool
                    zsbm = stm["zsb"]
                    wsb = w_pool.tile([128, NT, NB], F32R, tag="wsb")
                    stm["wsb"] = wsb
                    rrep_b = rrepE.unsqueeze(1).broadcast_to([128, 2, NB])
                    for w in range(NT // 2):
                        eng = nc.vector if w < 3 else nc.gpsimd
                        eng.tensor_tensor(
                            wsb[:, 2 * w : 2 * w + 2, :],
                            zsbm[:, 2 * w : 2 * w + 2, :],
                            rrep_b, mybir.AluOpType.mult,
                        )

                # ---- front(c): mm1 waves, z evac, squares, maha ----
                if has_front:
                    stf = state[c]
                    xtsf = stf["xts"]
                    stf["pm"] = pm
                    sq = sq_pool.tile([128, NT, NB], F32R, tag="sq")
                    zsb = z_pool.tile([128, NT, NB], F32R, tag="zsb")
                    stf["zsb"] = zsb
                    nc.tensor.matmul(pm, ct["H2c"], xtsf[0:64, :],
                                     start=True, stop=False)
                    for w in range(NT):
                        zw = zpsum.tile([128, NB], F32, tag="z")
                        nc.tensor.matmul(
                            zw, ct["A1s"][0:64, w, :], xtsf[0:64, :],
                            start=True, stop=True,
                        )
                        zs = zsb[:, w, :]
                        if w % 2 == 0:
                            nc.scalar.copy(out=zs, in_=zw)
                        else:
                            nc.vector.tensor_copy(zs, zw)
                        sqeng = nc.gpsimd if w < 6 else nc.vector
                        sqeng.tensor_tensor(
                            sq[:, w, :], zs, zs, mybir.AluOpType.mult
                        )
                        if w >= 2:
                            nc.tensor.matmul(
                                pm, ct["onesblk"], sq[:, w - 2, :],
                                start=False, stop=False,
                            )
                    for tt in (NT - 2, NT - 1):
                        nc.tensor.matmul(
                            pm, ct["onesblk"], sq[:, tt, :],
                            start=False, stop=False,
                        )
                    nc.tensor.matmul(pm, ct["cmm"], onesr, start=False,
                                     stop=True)

                # ---- transpose prefetch for chunk c+1 ----
                if has_tr:
                    xa = xin_pool.tile([128, NTILE, 64], F32R)
                    nc.sync.dma_start(out=xa, in_=xv[c + 1])
                    for j in range(NTILE):
                        nc.tensor.matmul(
                            r(pxt[:, j * 128 : (j + 1) * 128]), xa[:, j, :],
                            ct["ident"], is_transpose=True,
                        )
                    xts = xts_pool.tile([64, NB], F32R)
                    nc.scalar.copy(out=xts[0:64, :], in_=pxt)
                    state.setdefault(c + 1, {})["xts"] = xts

                # ---- back(c-2) finish: 1/S scale + DMA out ----
                if has_back:
                    sinvSB = small_pool.tile([64, NB], F32, tag="sinvSB")
                    nc.scalar.copy(out=sinvSB, in_=sinvB)
                    osb = o_pool.tile([64, NB], F32, tag="osb")
                    nc.vector.tensor_tensor(
                        osb, po, sinvSB, mybir.AluOpType.mult
                    )
                    nc.sync.dma_start(out=ovT[c - 2], in_=osb)

            def emit_transpose0():
                xa = xin_pool.tile([128, NTILE, 64], F32R)
                nc.sync.dma_start(out=xa, in_=xv[0])
                pxt0 = spsum.tile([64, NB], F32, tag="pxt")
                for j in range(NTILE):
                    nc.tensor.matmul(
                        r(pxt0[:, j * 128 : (j + 1) * 128]), xa[:, j, :],
                        ct["ident"], is_transpose=True,
                    )
                xts = xts_pool.tile([64, NB], F32R)
                nc.scalar.copy(out=xts[0:64, :], in_=pxt0)
                state.setdefault(0, {})["xts"] = xts

            loop_cm = (
                tc.For_i(0, repeat) if repeat else contextlib.nullcontext()
            )
            with loop_cm:
                emit_transpose0()
                for c in range(nchunk + 2):
                    emit_iter(c)

    return nc


def _legalize_waits(bir_bytes: bytes) -> bytes:
    """Walrus codegen allows at most ONE sync-wait per instruction. Tile's
    scheduler can emit several (one per upstream proc). Split the extras
    into standalone EventSemaphore instructions on the same engine, placed
    immediately before — the engine sequencer executes them in order, so
    semantics are preserved."""
    import json as _json

    bir = _json.loads(bir_bytes)
    n_new = 0
    for fn in bir["functions"]:
        for blk in fn["blocks"]:
            insts = blk.get("instructions", [])
            out = []
            for inst in insts:
                si = inst.get("sync_info")
                waits = (si or {}).get("on_wait") or []
                if len(waits) > 1:
                    for w in waits[:-1]:
                        n_new += 1
                        out.append({
                            "debug": inst.get("debug", 0),
                            "engine": inst["engine"],
                            "ins": [],
                            "name": f"I-waitsplit-{n_new}",
                            "opcode": "EventSemaphore",
                            "outs": [],
                            "sync_info": {"on_update": [], "on_wait": [w]},
                        })
                    si["on_wait"] = [waits[-1]]
                out.append(inst)
            blk["instructions"] = out
    return _json.dumps(bir).encode()


def _install_wait_legalizer():
    from concourse import bass2jax as _b2j
    from concourse import bass_utils as _bu

    if getattr(_b2j, "_wait_legalizer_installed", False):
        return
    _orig = _bu.compile_bir_kernel

    def _patched(bir_bytes, compile_dir_path, neff_name="file.neff", **kw):
        return _orig(_legalize_waits(bir_bytes), compile_dir_path,
                     neff_name=neff_name, **kw)

    _b2j.compile_bir_kernel = _patched
    _b2j._wait_legalizer_installed = True


_NC_CACHE = None


def kernel(x, means, weights, covs, alphas_cumprod, t):
    global _NC_CACHE
    x = np.ascontiguousarray(np.asarray(x, dtype=np.float32))
    consts = _host_precompute(
        np.asarray(means, dtype=np.float32),
        np.asarray(weights, dtype=np.float32),
        np.asarray(covs, dtype=np.float32),
        np.asarray(alphas_cumprod, dtype=np.float32),
        int(np.asarray(t)),
    )
    if _NC_CACHE is None:
        _NC_CACHE = _build_bass()
    nc = _NC_CACHE

    in_maps = []
    for c in range(NCORES):
        m = {"x_in": x[c * BP : (c + 1) * BP]}
        m.update(consts)
        in_maps.append(m)

    _install_wait_legalizer()
    res = run_bass_kernel_spmd(nc, in_maps, list(range(NCORES)))
    outs = [res.results[c]["outT"].T for c in range(NCORES)]
    return np.ascontiguousarray(np.concatenate(outs, axis=0), dtype=np.float32)


# revision 5
# speedup vs baseline: 1.1049x; 1.1049x over previous
"""Trainium2 Bass kernel (v13) for EpsilonNetGM score (8-core data parallel).

Math (closed-form score, no autodiff):
  acp = alphas_cumprod[t]; mu_k = sqrt(acp)*means_k
  Sigma_k = (1-acp) I + acp covs_k ; L = chol(Sigma); Linv = L^-1; P = Linv^T Linv
  z_k(x) = Linv_k x
  l_k(x) = -0.5|z_k|^2 + (P_k mu_k).x + c'_k      (c' <= 0, so exp never
  r~ = exp(l)  (UNNORMALIZED; l ~ -32 so no overflow/underflow risk)
  out = sqrt(1-acp)/S * [ sum_k Linv_k^T (r~_k z_k) - sum_k r~_k h_k ],
  S = sum_k r~_k

v8 key facts (measured on HW, not the CoreSim cost model):
  - fp32r matmul ~0.9ns/moving-row (full 128-partition out) + ~180ns fixed
    per instruction in-kernel; so NB=512 chunks halve instruction count
    and cut the PE stream 41% vs NB=256.
  - DVE tensor ops with BOTH operands in SBUF run ~2x faster (356ns/512el)
    than PSUM-reading ops (~650-720ns); Pool (gpsimd) works at ~1049ns.
  - So: z is computed once, evacuated PSUM->SBUF, squares (Pool+DVE) and
    W=z*r (all DVE; Pool W pairs measured slower in situ) are SBUF-SBUF
    TTs, and there is no z recompute. W runs a full pipeline stage
    before its mm2.

Pipeline per emission iteration c (NCHUNK=16 chunks of 512 rows):
  back(c-2) PE first (zero-gated: wsb finished last iter), then mid(c-1)
  (exp^T/replicate/sum/recip + W), then front(c) waves (mm1 -> zevac ->
  square -> trailing maha), transpose prefetch (c+1), and back(c-2)
  finish (1/S scale + DMA) last.
"""

import math
import sys

import numpy as np

sys.path.insert(0, "/opt/trn_rl_repo")

import concourse.bass as bass  # noqa: E402
import concourse.tile as tile  # noqa: E402
from concourse import mybir  # noqa: E402
from concourse.bass_utils import run_bass_kernel_spmd  # noqa: E402

B, K, D, T = 65536, 16, 64, 1000
NCORES = 8
BP = B // NCORES          # rows per core = 8192
NB = 512                  # batch chunk (free dim)
NTILE = 4                 # 128-row tiles per chunk
NCHUNK = BP // NB         # 16
DS = 8                    # d-subtile width; partition p = 8*k + ds
NT = D // DS              # 8 subtiles
CW = 2432                 # const blob width

F32 = mybir.dt.float32
F32R = mybir.dt.float32r


def _host_precompute(means, weights, covs, alphas_cumprod, t):
    acp = float(np.asarray(alphas_cumprod)[int(t)])
    s1 = math.sqrt(acp)
    sqrt1m = math.sqrt(1.0 - acp)
    mu = (s1 * means).astype(np.float64)
    covs = covs.astype(np.float64)
    sigma = (1.0 - acp) * np.eye(D) + acp * covs
    chol = np.linalg.cholesky(sigma)
    Linv = np.stack([np.linalg.solve(chol[k], np.eye(D)) for k in range(K)])
    P = np.einsum("kdi,kdj->kij", Linv, Linv)
    h = np.einsum("kij,kj->ki", P, mu)
    logdet = 2.0 * np.log(np.diagonal(chol, axis1=1, axis2=2)).sum(-1)
    w = weights.astype(np.float64)
    logw = np.log(w) - math.log(w.sum())
    c = logw - 0.5 * (D * math.log(2 * math.pi) + logdet)
    cp = c - 0.5 * np.einsum("ki,ki->k", mu, h)
    cp = cp - cp.max()

    # A1s [128, NT, 128]: rows d' (dup 0-63/64-127), col p = 8k+ds
    A1 = np.zeros((64, NT, 128), dtype=np.float32)
    A2s = np.zeros((128, NT, 64), dtype=np.float32)
    for k in range(K):
        for ds in range(DS):
            p = 8 * k + ds
            for tt in range(NT):
                A1[:, tt, p] = Linv[k, 8 * tt + ds, :]
                A2s[p, tt, :] = sqrt1m * Linv[k, 8 * tt + ds, :]
    A1s = np.concatenate([A1, A1], axis=0)

    onesblk = np.zeros((128, K), dtype=np.float32)
    for k in range(K):
        onesblk[8 * k : 8 * k + 8, k] = -0.5  # fold -0.5 into the reduce
    cmm = cp.astype(np.float32).reshape(1, K)
    ERep = np.zeros((K, 128), dtype=np.float32)
    for k in range(K):
        ERep[k, 8 * k : 8 * k + 8] = 1.0
    negHs = (-sqrt1m * h).astype(np.float32)   # [K, 64]
    H2c = h.T.astype(np.float32)               # [64, K]
    ident = np.eye(128, dtype=np.float32)

    blob = np.zeros((128, CW), dtype=np.float32)
    blob[:, 0:1024] = A1s.reshape(128, 1024)
    blob[:, 1024:1536] = A2s.reshape(128, 512)
    blob[:, 1536:1552] = onesblk
    blob[:, 1552:1680] = ident
    blob[0, 1680:1696] = cmm[0]
    blob[0:16, 1696:1824] = ERep
    blob[0:16, 1824:1888] = negHs
    blob[0:64, 1888:1904] = H2c
    blob[0, 1904 : 1904 + NB] = 1.0   # ones row [1, NB]
    blob[0:16, 2416] = 1.0            # ones column [16, 1] for the k-sum
    return dict(cblob=blob)


def _build_bass(nchunk=NCHUNK, repeat=None):
    """repeat=R wraps the chunk loop in a hardware For_i for timing builds."""
    import contextlib

    nc = bass.Bass()
    x_in = nc.declare_dram_parameter("x_in", [BP, D], F32R, isOutput=False)
    outT = nc.declare_dram_parameter("outT", [D, BP], F32, isOutput=True)
    c_blob = nc.declare_dram_parameter("cblob", [128, CW], F32R, isOutput=False)

    xv = x_in.rearrange("(n j p) d -> n p j d", p=128, j=NTILE)
    ovT = outT.rearrange("d (n b) -> n d b", b=NB)

    r = lambda ap: ap.bitcast(F32R)  # noqa: E731

    with tile.TileContext(nc) as tc:
        with (
            tc.tile_pool(name="consts", bufs=1) as consts,
            tc.tile_pool(name="xin", bufs=3) as xin_pool,
            tc.tile_pool(name="xts", bufs=2) as xts_pool,
            tc.tile_pool(name="zpsum", bufs=2, space="PSUM") as zpsum,
            tc.tile_pool(name="spsum", bufs=1, space="PSUM") as spsum,
            tc.tile_pool(name="sq", bufs=2) as sq_pool,
            tc.tile_pool(name="zsb", bufs=2) as z_pool,
            tc.tile_pool(name="small", bufs=3) as small_pool,
            tc.tile_pool(name="wbuf", bufs=2) as w_pool,
            tc.tile_pool(name="obuf", bufs=2) as o_pool,
        ):
            cblob = consts.tile([128, CW], F32R)
            nc.sync.dma_start(out=cblob, in_=c_blob[...])
            ct = {
                "A1s": cblob[:, 0:1024].rearrange("p (t c) -> p t c", t=NT),
                "A2s": cblob[:, 1024:1536].rearrange("p (t c) -> p t c", t=NT),
                "onesblk": cblob[:, 1536:1552],
                "ident": cblob[:, 1552:1680],
                "cmm": cblob[0:1, 1680:1696],
                "ERep": cblob[0:16, 1696:1824],
                "negHs": cblob[0:16, 1824:1888],
                "H2c": cblob[0:64, 1888:1904],
            }
            onesr = cblob[0:1, 1904 : 1904 + NB]
            ones64 = cblob[0:1, 1904 : 1904 + 64]
            ones16 = cblob[0:16, 2416:2417]

            # PE warmup read of cblob so later matmuls don't each need a
            # DMA wait (walrus allows only one sync-wait per instruction).
            pwarm = zpsum.tile([128, NB], F32, tag="z")
            nc.tensor.matmul(
                pwarm[0:32, 0:32].bitcast(F32R), ct["ident"][0:32, 0:32],
                ct["ident"][0:32, 0:32], is_transpose=True,
            )

            state = {}

            def emit_iter(c):
                has_front = c < nchunk
                has_tr = c + 1 < nchunk
                has_mid = 1 <= c <= nchunk
                has_back = c >= 2

                # single-buffered PSUM banks (one per role; all roles are
                # intra-iteration except pm which is read early next iter)
                pm = spsum.tile([K, NB], F32, tag="pm")
                psumS = spsum.tile([1, NB], F32, tag="psumS")
                pxt = spsum.tile([64, NB], F32, tag="pxt")
                po = spsum.tile([64, NB], F32, tag="po")
                prep = spsum.tile([128, NB], F32, tag="prep")
                sinvB = spsum.tile([64, NB], F32, tag="sinvB")

                # ---- back(c-2) PE section first: fully slack-fed ----
                if has_back:
                    stb = state.pop(c - 2)
                    eeTb, sinvb, wsbb = stb["eeT"], stb["sinv"], stb["wsb"]
                    nc.tensor.matmul(po, ct["negHs"], eeTb, start=True,
                                     stop=False)
                    for tt in range(NT):
                        nc.tensor.matmul(
                            po, ct["A2s"][:, tt, :], wsbb[:, tt, :],
                            start=False, stop=(tt == NT - 1),
                        )
                    nc.tensor.matmul(sinvB, ones64, sinvb, start=True,
                                     stop=True)

                # ---- mid(c-1): exp^T, replicate, k-sum, recip, W ----
                if has_mid:
                    stm = state[c - 1]
                    pmm = stm["pm"]
                    eeT = small_pool.tile([K, NB], F32R, tag="eeT")
                    nc.scalar.activation(
                        eeT, pmm,
                        mybir.ActivationFunctionType.Exp,
                        bias=0.0, scale=1.0,
                    )
                    rrepE = small_pool.tile([128, NB], F32, tag="rrepE")
                    nc.tensor.matmul(prep, ct["ERep"], eeT, start=True,
                                     stop=True)
                    nc.scalar.copy(out=rrepE, in_=prep)
                    nc.tensor.matmul(psumS, ones16, eeT, start=True,
                                     stop=True)
                    sinv = small_pool.tile([1, NB], F32R, tag="sinv")
                    with nc.allow_low_precision(
                        reason="f32r is full fp32 width; tag only gates PE"
                    ):
                        nc.vector.reciprocal(sinv, psumS)
                    stm["eeT"] = eeT
                    stm["sinv"] = sinv
                    # W(c-1) = zsb * r~rep: SBUF-SBUF TTs, one stage ahead
                    # of mm2; split DVE/Pool
                    zsbm = stm["zsb"]
                    wsb = w_pool.tile([128, NT, NB], F32R, tag="wsb")
                    stm["wsb"] = wsb
                    rrep_b = rrepE.unsqueeze(1).broadcast_to([128, 2, NB])
                    for w in range(NT // 2):
                        eng = nc.vector
                        eng.tensor_tensor(
                            wsb[:, 2 * w : 2 * w + 2, :],
                            zsbm[:, 2 * w : 2 * w + 2, :],
                            rrep_b, mybir.AluOpType.mult,
                        )

                # ---- front(c): mm1 waves, z evac, squares, maha ----
                if has_front:
                    stf = state[c]
                    xtsf = stf["xts"]
                    stf["pm"] = pm
                    sq = sq_pool.tile([128, NT, NB], F32R, tag="sq")
                    zsb = z_pool.tile([128, NT, NB], F32R, tag="zsb")
                    stf["zsb"] = zsb
                    nc.tensor.matmul(pm, ct["H2c"], xtsf[0:64, :],
                                     start=True, stop=False)
                    for w in range(NT):
                        zw = zpsum.tile([128, NB], F32, tag="z")
                        nc.tensor.matmul(
                            zw, ct["A1s"][0:64, w, :], xtsf[0:64, :],
                            start=True, stop=True,
                        )
                        zs = zsb[:, w, :]
                        if w % 2 == 0:
                            nc.scalar.copy(out=zs, in_=zw)
                        else:
                            nc.vector.tensor_copy(zs, zw)
                        sqeng = nc.gpsimd if w < 6 else nc.vector
                        sqeng.tensor_tensor(
                            sq[:, w, :], zs, zs, mybir.AluOpType.mult
                        )
                        if w >= 2:
                            nc.tensor.matmul(
                                pm, ct["onesblk"], sq[:, w - 2, :],
                                start=False, stop=False,
                            )
                    for tt in (NT - 2, NT - 1):
                        nc.tensor.matmul(
                            pm, ct["onesblk"], sq[:, tt, :],
                            start=False, stop=False,
                        )
                    nc.tensor.matmul(pm, ct["cmm"], onesr, start=False,
                                     stop=True)

                # ---- transpose prefetch for chunk c+1 ----
                if has_tr:
                    xa = xin_pool.tile([128, NTILE, 64], F32R)
                    nc.sync.dma_start(out=xa, in_=xv[c + 1])
                    for j in range(NTILE):
                        nc.tensor.matmul(
                            r(pxt[:, j * 128 : (j + 1) * 128]), xa[:, j, :],
                            ct["ident"], is_transpose=True,
                        )
                    xts = xts_pool.tile([64, NB], F32R)
                    nc.scalar.copy(out=xts[0:64, :], in_=pxt)
                    state.setdefault(c + 1, {})["xts"] = xts

                # ---- back(c-2) finish: 1/S scale + DMA out ----
                if has_back:
                    sinvSB = small_pool.tile([64, NB], F32, tag="sinvSB")
                    nc.scalar.copy(out=sinvSB, in_=sinvB)
                    osb = o_pool.tile([64, NB], F32, tag="osb")
                    nc.vector.tensor_tensor(
                        osb, po, sinvSB, mybir.AluOpType.mult
                    )
                    nc.sync.dma_start(out=ovT[c - 2], in_=osb)

            def emit_transpose0():
                xa = xin_pool.tile([128, NTILE, 64], F32R)
                nc.sync.dma_start(out=xa, in_=xv[0])
                pxt0 = spsum.tile([64, NB], F32, tag="pxt")
                for j in range(NTILE):
                    nc.tensor.matmul(
                        r(pxt0[:, j * 128 : (j + 1) * 128]), xa[:, j, :],
                        ct["ident"], is_transpose=True,
                    )
                xts = xts_pool.tile([64, NB], F32R)
                nc.scalar.copy(out=xts[0:64, :], in_=pxt0)
                state.setdefault(0, {})["xts"] = xts

            loop_cm = (
                tc.For_i(0, repeat) if repeat else contextlib.nullcontext()
            )
            with loop_cm:
                emit_transpose0()
                for c in range(nchunk + 2):
                    emit_iter(c)

    return nc


def _legalize_waits(bir_bytes: bytes) -> bytes:
    """Walrus codegen allows at most ONE sync-wait per instruction. Tile's
    scheduler can emit several (one per upstream proc). Split the extras
    into standalone EventSemaphore instructions on the same engine, placed
    immediately before — the engine sequencer executes them in order, so
    semantics are preserved."""
    import json as _json

    bir = _json.loads(bir_bytes)
    n_new = 0
    for fn in bir["functions"]:
        for blk in fn["blocks"]:
            insts = blk.get("instructions", [])
            out = []
            for inst in insts:
                si = inst.get("sync_info")
                waits = (si or {}).get("on_wait") or []
                if len(waits) > 1:
                    for w in waits[:-1]:
                        n_new += 1
                        out.append({
                            "debug": inst.get("debug", 0),
                            "engine": inst["engine"],
                            "ins": [],
                            "name": f"I-waitsplit-{n_new}",
                            "opcode": "EventSemaphore",
                            "outs": [],
                            "sync_info": {"on_update": [], "on_wait": [w]},
                        })
                    si["on_wait"] = [waits[-1]]
                out.append(inst)
            blk["instructions"] = out
    return _json.dumps(bir).encode()


def _install_wait_legalizer():
    from concourse import bass2jax as _b2j
    from concourse import bass_utils as _bu

    if getattr(_b2j, "_wait_legalizer_installed", False):
        return
    _orig = _bu.compile_bir_kernel

    def _patched(bir_bytes, compile_dir_path, neff_name="file.neff", **kw):
        return _orig(_legalize_waits(bir_bytes), compile_dir_path,
                     neff_name=neff_name, **kw)

    _b2j.compile_bir_kernel = _patched
    _b2j._wait_legalizer_installed = True


_NC_CACHE = None


def kernel(x, means, weights, covs, alphas_cumprod, t):
    global _NC_CACHE
    x = np.ascontiguousarray(np.asarray(x, dtype=np.float32))
    consts = _host_precompute(
        np.asarray(means, dtype=np.float32),
        np.asarray(weights, dtype=np.float32),
        np.asarray(covs, dtype=np.float32),
        np.asarray(alphas_cumprod, dtype=np.float32),
        int(np.asarray(t)),
    )
    if _NC_CACHE is None:
        _NC_CACHE = _build_bass()
    nc = _NC_CACHE

    in_maps = []
    for c in range(NCORES):
        m = {"x_in": x[c * BP : (c + 1) * BP]}
        m.update(consts)
        in_maps.append(m)

    _install_wait_legalizer()
    res = run_bass_kernel_spmd(nc, in_maps, list(range(NCORES)))
    outs = [res.results[c]["outT"].T for c in range(NCORES)]
    return np.ascontiguousarray(np.concatenate(outs, axis=0), dtype=np.float32)


# revision 6
# speedup vs baseline: 1.1619x; 1.0516x over previous
"""Trainium2 Bass kernel v17 for EpsilonNetGM score (8-core data parallel).

Math (closed-form score, no autodiff):
  acp = alphas_cumprod[t]; mu_k = sqrt(acp)*means_k
  Sigma_k = (1-acp) I + acp covs_k ; L = chol(Sigma); Linv = L^-1; P = Linv^T Linv
  z_k(x) = Linv_k x
  l_k(x) = -0.5|z_k|^2 + (P_k mu_k).x + c'_k      (c' <= 0, so exp never
  r~ = exp(l)  (UNNORMALIZED; l ~ -32 so no overflow/underflow risk)
  out = sqrt(1-acp)/S * [ sum_k Linv_k^T (r~_k z_k) - sum_k r~_k h_k ],
  S = sum_k r~_k

v8 key facts (measured on HW, not the CoreSim cost model):
  - fp32r matmul ~0.9ns/moving-row (full 128-partition out) + ~180ns fixed
    per instruction in-kernel; so NB=512 chunks halve instruction count
    and cut the PE stream 41% vs NB=256.
  - DVE tensor ops with BOTH operands in SBUF run ~2x faster (356ns/512el)
    than PSUM-reading ops (~650-720ns); Pool (gpsimd) works at ~1049ns.
  - So: z is computed once, evacuated PSUM->SBUF, squares and W=z*r are
    cheap SBUF-SBUF TTs spread across Pool/DVE, and there is no z
    recompute. W runs a full pipeline stage before its mm2.

Pipeline per emission iteration c (NCHUNK=16 chunks of 512 rows):
  back(c-2) PE first (zero-gated: wsb finished last iter), then mid(c-1)
  (exp^T/replicate/sum/recip + W), then front(c) waves (mm1 -> zevac ->
  square -> trailing maha), transpose prefetch (c+1), and back(c-2)
  finish (1/S scale + DMA) last.
"""

import math
import sys

import numpy as np

sys.path.insert(0, "/opt/trn_rl_repo")

import concourse.bass as bass  # noqa: E402
import concourse.tile as tile  # noqa: E402
from concourse import mybir  # noqa: E402
from concourse.bass_utils import run_bass_kernel_spmd  # noqa: E402

B, K, D, T = 65536, 16, 64, 1000
NCORES = 8
BP = B // NCORES          # rows per core = 8192
NB = 512                  # batch chunk (free dim)
NTILE = 4                 # 128-row tiles per chunk
NCHUNK = BP // NB         # 16
DS = 8                    # d-subtile width; partition p = 8*k + ds
NT = D // DS              # 8 subtiles
CW = 2432                 # const blob width

F32 = mybir.dt.float32
F32R = mybir.dt.float32r


def _host_precompute(means, weights, covs, alphas_cumprod, t):
    acp = float(np.asarray(alphas_cumprod)[int(t)])
    s1 = math.sqrt(acp)
    sqrt1m = math.sqrt(1.0 - acp)
    mu = (s1 * means).astype(np.float64)
    covs = covs.astype(np.float64)
    sigma = (1.0 - acp) * np.eye(D) + acp * covs
    chol = np.linalg.cholesky(sigma)
    Linv = np.stack([np.linalg.solve(chol[k], np.eye(D)) for k in range(K)])
    P = np.einsum("kdi,kdj->kij", Linv, Linv)
    h = np.einsum("kij,kj->ki", P, mu)
    logdet = 2.0 * np.log(np.diagonal(chol, axis1=1, axis2=2)).sum(-1)
    w = weights.astype(np.float64)
    logw = np.log(w) - math.log(w.sum())
    c = logw - 0.5 * (D * math.log(2 * math.pi) + logdet)
    cp = c - 0.5 * np.einsum("ki,ki->k", mu, h)
    cp = cp - cp.max()

    # A1s [128, NT, 128]: rows d' (dup 0-63/64-127), col p = 8k+ds
    A1 = np.zeros((64, NT, 128), dtype=np.float32)
    A2s = np.zeros((128, NT, 64), dtype=np.float32)
    for k in range(K):
        for ds in range(DS):
            p = 8 * k + ds
            for tt in range(NT):
                A1[:, tt, p] = Linv[k, 8 * tt + ds, :]
                A2s[p, tt, :] = sqrt1m * Linv[k, 8 * tt + ds, :]
    A1s = np.concatenate([A1, A1], axis=0)

    onesblk = np.zeros((128, K), dtype=np.float32)
    for k in range(K):
        onesblk[8 * k : 8 * k + 8, k] = -0.5  # fold -0.5 into the reduce
    cmm = cp.astype(np.float32).reshape(1, K)
    ERep = np.zeros((K, 128), dtype=np.float32)
    for k in range(K):
        ERep[k, 8 * k : 8 * k + 8] = 1.0
    negHs = (-sqrt1m * h).astype(np.float32)   # [K, 64]
    H2c = h.T.astype(np.float32)               # [64, K]
    ident = np.eye(128, dtype=np.float32)

    blob = np.zeros((128, CW), dtype=np.float32)
    blob[:, 0:1024] = A1s.reshape(128, 1024)
    blob[:, 1024:1536] = A2s.reshape(128, 512)
    blob[:, 1536:1552] = onesblk
    blob[:, 1552:1680] = ident
    blob[0, 1680:1696] = cmm[0]
    blob[0:16, 1696:1824] = ERep
    blob[0:16, 1824:1888] = negHs
    blob[0:64, 1888:1904] = H2c
    blob[0, 1904 : 1904 + NB] = 1.0   # ones row [1, NB]
    blob[0:16, 2416] = 1.0            # ones column [16, 1] for the k-sum
    return dict(cblob=blob)


def _build_bass(nchunk=NCHUNK, repeat=None):
    """repeat=R wraps the chunk loop in a hardware For_i for timing builds."""
    import contextlib

    nc = bass.Bass()
    x_in = nc.declare_dram_parameter("x_in", [BP, D], F32R, isOutput=False)
    outT = nc.declare_dram_parameter("outT", [D, BP], F32, isOutput=True)
    c_blob = nc.declare_dram_parameter("cblob", [128, CW], F32R, isOutput=False)

    xv = x_in.rearrange("(n j p) d -> n p j d", p=128, j=NTILE)
    ovT = outT.rearrange("d (n b) -> n d b", b=NB)

    r = lambda ap: ap.bitcast(F32R)  # noqa: E731

    with tile.TileContext(nc) as tc:
        with (
            tc.tile_pool(name="consts", bufs=1) as consts,
            tc.tile_pool(name="xin", bufs=3) as xin_pool,
            tc.tile_pool(name="xts", bufs=2) as xts_pool,
            tc.tile_pool(name="zpsum", bufs=2, space="PSUM") as zpsum,
            tc.tile_pool(name="spsum", bufs=1, space="PSUM") as spsum,
            tc.tile_pool(name="sq", bufs=2) as sq_pool,
            tc.tile_pool(name="zsb", bufs=2) as z_pool,
            tc.tile_pool(name="small", bufs=3) as small_pool,
            tc.tile_pool(name="wbuf", bufs=2) as w_pool,
            tc.tile_pool(name="obuf", bufs=2) as o_pool,
        ):
            cblob = consts.tile([128, CW], F32R)
            nc.sync.dma_start(out=cblob, in_=c_blob[...])
            ct = {
                "A1s": cblob[:, 0:1024].rearrange("p (t c) -> p t c", t=NT),
                "A2s": cblob[:, 1024:1536].rearrange("p (t c) -> p t c", t=NT),
                "onesblk": cblob[:, 1536:1552],
                "ident": cblob[:, 1552:1680],
                "cmm": cblob[0:1, 1680:1696],
                "ERep": cblob[0:16, 1696:1824],
                "negHs": cblob[0:16, 1824:1888],
                "H2c": cblob[0:64, 1888:1904],
            }
            onesr = cblob[0:1, 1904 : 1904 + NB]
            ones64 = cblob[0:1, 1904 : 1904 + 64]
            ones16 = cblob[0:16, 2416:2417]

            # PE warmup read of cblob so later matmuls don't each need a
            # DMA wait (walrus allows only one sync-wait per instruction).
            pwarm = zpsum.tile([128, 2, NB], F32, tag="z")
            nc.tensor.matmul(
                pwarm[0:32, 0, 0:32].bitcast(F32R), ct["ident"][0:32, 0:32],
                ct["ident"][0:32, 0:32], is_transpose=True,
            )

            state = {}

            def emit_iter(c):
                has_front = c < nchunk
                has_tr = c + 1 < nchunk
                has_mid = 1 <= c <= nchunk
                has_back = c >= 2

                # pm and prep own a bank each; sinvB -> po -> psumS -> pxt
                # have strictly sequential lifetimes inside one iteration,
                # so they share one rotating 2-buffer tag (2 banks). The z
                # pool holds 2-bank pair tiles x2 (4 banks). Total = 8.
                pm = spsum.tile([K, NB], F32, tag="pm")
                prep = spsum.tile([128, NB], F32, tag="prep")

                # ---- back(c-2): PE section + 1/S scale + DMA, first ----
                if has_back:
                    stb = state.pop(c - 2)
                    eeTb, sinvb, wsbb = stb["eeT"], stb["sinv"], stb["wsb"]
                    sinvB = spsum.tile([64, NB], F32, tag="sh64",
                                       bufs=2, name="sinvB")
                    nc.tensor.matmul(sinvB, ones64, sinvb, start=True,
                                     stop=True)
                    sinvSB = small_pool.tile([64, NB], F32, tag="sinvSB")
                    nc.scalar.copy(out=sinvSB, in_=sinvB)
                    po = spsum.tile([64, NB], F32, tag="sh64", bufs=2, name="po")
                    nc.tensor.matmul(po, ct["negHs"], eeTb, start=True,
                                     stop=False)
                    for tt in range(NT):
                        nc.tensor.matmul(
                            po, ct["A2s"][:, tt, :], wsbb[:, tt, :],
                            start=False, stop=(tt == NT - 1),
                        )
                    osb = o_pool.tile([64, NB], F32, tag="osb")
                    nc.vector.tensor_tensor(
                        osb, po, sinvSB, mybir.AluOpType.mult
                    )
                    nc.sync.dma_start(out=ovT[c - 2], in_=osb)

                # ---- mid(c-1): exp^T, replicate, k-sum, recip ----
                if has_mid:
                    stm = state[c - 1]
                    pmm = stm["pm"]
                    eeT = small_pool.tile([K, NB], F32R, tag="eeT")
                    nc.scalar.activation(
                        eeT, pmm,
                        mybir.ActivationFunctionType.Exp,
                        bias=0.0, scale=1.0,
                    )
                    rrepE = small_pool.tile([128, NB], F32, tag="rrepE")
                    nc.tensor.matmul(prep, ct["ERep"], eeT, start=True,
                                     stop=True)
                    nc.scalar.copy(out=rrepE, in_=prep)
                    psumS = spsum.tile([1, NB], F32, tag="sh64", bufs=2,
                                       name="psumS")
                    nc.tensor.matmul(psumS, ones16, eeT, start=True,
                                     stop=True)
                    sinv = small_pool.tile([1, NB], F32R, tag="sinv")
                    with nc.allow_low_precision(
                        reason="f32r is full fp32 width; tag only gates PE"
                    ):
                        nc.vector.reciprocal(sinv, psumS)
                    stm["eeT"] = eeT
                    stm["sinv"] = sinv
                    zsbm = stm["zsb"]
                    wsb = w_pool.tile([128, NT, NB], F32R, tag="wsb")
                    stm["wsb"] = wsb
                    rrep_b = rrepE.unsqueeze(1).broadcast_to([128, 2, NB])

                    def emit_w_pair(w):
                        nc.vector.tensor_tensor(
                            wsb[:, 2 * w : 2 * w + 2, :],
                            zsbm[:, 2 * w : 2 * w + 2, :],
                            rrep_b, mybir.AluOpType.mult,
                        )
                else:
                    emit_w_pair = None

                # ---- front(c): paired mm1 waves, z evac, squares, maha;
                # W pairs of chunk c-1 interleave with the waves ----
                if has_front:
                    stf = state[c]
                    xtsf = stf["xts"]
                    stf["pm"] = pm
                    sq = sq_pool.tile([128, NT, NB], F32R, tag="sq")
                    zsb = z_pool.tile([128, NT, NB], F32R, tag="zsb")
                    stf["zsb"] = zsb
                    nc.tensor.matmul(pm, ct["H2c"], xtsf[0:64, :],
                                     start=True, stop=False)
                    for w in range(NT // 2):
                        zw = zpsum.tile([128, 2, NB], F32, tag="z")
                        for h in range(2):
                            nc.tensor.matmul(
                                zw[:, h, :],
                                ct["A1s"][0:64, 2 * w + h, :],
                                xtsf[0:64, :],
                                start=True, stop=True,
                            )
                        zs = zsb[:, 2 * w : 2 * w + 2, :]
                        if w % 2 == 0:
                            nc.scalar.copy(out=zs, in_=zw)
                        else:
                            nc.vector.tensor_copy(zs, zw)
                        sqeng = nc.gpsimd if w < 3 else nc.vector
                        sqeng.tensor_tensor(
                            sq[:, 2 * w : 2 * w + 2, :], zs, zs,
                            mybir.AluOpType.mult,
                        )
                        if emit_w_pair is not None:
                            emit_w_pair(w)
                        if w >= 2:
                            for tt in (2 * (w - 2), 2 * (w - 2) + 1):
                                nc.tensor.matmul(
                                    pm, ct["onesblk"], sq[:, tt, :],
                                    start=False, stop=False,
                                )
                    for tt in range(NT - 4, NT):
                        nc.tensor.matmul(
                            pm, ct["onesblk"], sq[:, tt, :],
                            start=False, stop=False,
                        )
                    nc.tensor.matmul(pm, ct["cmm"], onesr, start=False,
                                     stop=True)
                elif emit_w_pair is not None:
                    for w in range(NT // 2):
                        emit_w_pair(w)

                # ---- transpose prefetch for chunk c+1 ----
                if has_tr:
                    xa = xin_pool.tile([128, NTILE, 64], F32R)
                    nc.sync.dma_start(out=xa, in_=xv[c + 1])
                    pxt = spsum.tile([64, NB], F32, tag="sh64", bufs=2, name="pxt")
                    for j in range(NTILE):
                        nc.tensor.matmul(
                            r(pxt[:, j * 128 : (j + 1) * 128]), xa[:, j, :],
                            ct["ident"], is_transpose=True,
                        )
                    xts = xts_pool.tile([64, NB], F32R)
                    nc.scalar.copy(out=xts[0:64, :], in_=pxt)
                    state.setdefault(c + 1, {})["xts"] = xts

            def emit_transpose0():
                xa = xin_pool.tile([128, NTILE, 64], F32R)
                nc.sync.dma_start(out=xa, in_=xv[0])
                pxt0 = spsum.tile([64, NB], F32, tag="sh64", bufs=2, name="pxt0")
                for j in range(NTILE):
                    nc.tensor.matmul(
                        r(pxt0[:, j * 128 : (j + 1) * 128]), xa[:, j, :],
                        ct["ident"], is_transpose=True,
                    )
                xts = xts_pool.tile([64, NB], F32R)
                nc.scalar.copy(out=xts[0:64, :], in_=pxt0)
                state.setdefault(0, {})["xts"] = xts

            loop_cm = (
                tc.For_i(0, repeat) if repeat else contextlib.nullcontext()
            )
            with loop_cm:
                emit_transpose0()
                for c in range(nchunk + 2):
                    emit_iter(c)

    return nc


def _legalize_waits(bir_bytes: bytes) -> bytes:
    """Walrus codegen allows at most ONE sync-wait per instruction. Tile's
    scheduler can emit several (one per upstream proc). Split the extras
    into standalone EventSemaphore instructions on the same engine, placed
    immediately before — the engine sequencer executes them in order, so
    semantics are preserved."""
    import json as _json

    bir = _json.loads(bir_bytes)
    n_new = 0
    for fn in bir["functions"]:
        for blk in fn["blocks"]:
            insts = blk.get("instructions", [])
            out = []
            for inst in insts:
                si = inst.get("sync_info")
                waits = (si or {}).get("on_wait") or []
                if len(waits) > 1:
                    for w in waits[:-1]:
                        n_new += 1
                        out.append({
                            "debug": inst.get("debug", 0),
                            "engine": inst["engine"],
                            "ins": [],
                            "name": f"I-waitsplit-{n_new}",
                            "opcode": "EventSemaphore",
                            "outs": [],
                            "sync_info": {"on_update": [], "on_wait": [w]},
                        })
                    si["on_wait"] = [waits[-1]]
                out.append(inst)
            blk["instructions"] = out
    return _json.dumps(bir).encode()


def _install_wait_legalizer():
    from concourse import bass2jax as _b2j
    from concourse import bass_utils as _bu

    if getattr(_b2j, "_wait_legalizer_installed", False):
        return
    _orig = _bu.compile_bir_kernel

    def _patched(bir_bytes, compile_dir_path, neff_name="file.neff", **kw):
        return _orig(_legalize_waits(bir_bytes), compile_dir_path,
                     neff_name=neff_name, **kw)

    _b2j.compile_bir_kernel = _patched
    _b2j._wait_legalizer_installed = True


_NC_CACHE = None


def kernel(x, means, weights, covs, alphas_cumprod, t):
    global _NC_CACHE
    x = np.ascontiguousarray(np.asarray(x, dtype=np.float32))
    consts = _host_precompute(
        np.asarray(means, dtype=np.float32),
        np.asarray(weights, dtype=np.float32),
        np.asarray(covs, dtype=np.float32),
        np.asarray(alphas_cumprod, dtype=np.float32),
        int(np.asarray(t)),
    )
    if _NC_CACHE is None:
        _NC_CACHE = _build_bass()
    nc = _NC_CACHE

    in_maps = []
    for c in range(NCORES):
        m = {"x_in": x[c * BP : (c + 1) * BP]}
        m.update(consts)
        in_maps.append(m)

    _install_wait_legalizer()
    res = run_bass_kernel_spmd(nc, in_maps, list(range(NCORES)))
    outs = [res.results[c]["outT"].T for c in range(NCORES)]
    return np.ascontiguousarray(np.concatenate(outs, axis=0), dtype=np.float32)
